# revision 6
# baseline (speedup 1.0000x reference)
"""Trainium2 Bass kernel v2 for EquivariantSelfAttention (B=4, N=2048, HID=256, 8 heads).

Sharding: 8 cores = 4 batches x 2 query-halves (1024 queries/core, full keys).

Attention loop: 8 iterations of (head-pair p, query-half qh) x 16 key tiles.
Per kt: 2 score MMs -> ss [128 keys, 2h*512q] f32 (2 banks, double-buffered),
one exp (FD=1024) on ScalarE, then 12 PV MMs col-packed into 3 accumulator
banks (row-shared regions, incl. a duplicated ones-matmul denominator so a
single reciprocal + aligned tensor_tensor ops normalize everything).
ScalarE streams exps back-to-back; DVE/PE/GpSimd work hides underneath.
"""

import sys

if "/opt/trn_rl_repo" not in sys.path:
    sys.path.insert(0, "/opt/trn_rl_repo")

import numpy as np
import ml_dtypes

B, N, HID, NH, HD = 4, 2048, 256, 8, 32
NQ = N // 2          # queries per core
NKT = N // 128       # key tiles
SCALE = float(1.0 / np.sqrt(HD))
BF = ml_dtypes.bfloat16

_CACHE = {}


def _build_nc():
    import concourse.bass as bass
    import concourse.mybir as mybir
    import concourse.tile as tile
    from concourse import bacc
    from concourse.bass import ts

    f32 = mybir.dt.float32
    bf16 = mybir.dt.bfloat16
    AF = mybir.ActivationFunctionType
    OP = mybir.AluOpType
    P = 128

    nc = bacc.Bacc("TRN2", target_bir_lowering=False, debug=False,
                   enable_asserts=False, num_devices=8)

    def din(name, shape, dt):
        return nc.dram_tensor(name, shape, dt, kind="ExternalInput").ap()

    # wm columns: wq(512) wk(512) wv(512) wvec(1024) woP2(1536) wgP(2048)
    #             ones(128)  => 6272
    WM_COLS = 6272
    # bm columns: bq(2) bk(2) bo(6) bgP(4) bvB4(1024 f32) => 1038
    BM_COLS = 1038
    xm = din("xm", [P, 2 * N], bf16)    # xsT (query-half tokens first)
    wm = din("wm", [P, WM_COLS], bf16)
    vvec = din("vvec", [P, NKT * 768], bf16)         # vec token tiles (c,h,d)
    vq16m = din("vq16m", [P, 6 * NQ], bf16)
    vqAB = din("vqAB", [P, 4 * NQ], bf16)             # per-pair c0|c1 rows
    vqC2 = din("vqC2", [P, 2 * NQ], bf16)             # per pair-pair c2 rows
    bm = din("bm", [P, BM_COLS], f32)
    out = nc.dram_tensor("out", [4 * HID, NQ], bf16, kind="ExternalOutput").ap()

    with tile.TileContext(nc) as tc:
        from contextlib import ExitStack
        with ExitStack() as ctx:
            def sb(name, shape, dt):
                return nc.alloc_sbuf_tensor("sb_" + name, list(shape), dt).ap()

            # ---------------- persistent SBUF ----------------
            xm_s = sb("xm", [P, 2 * N], bf16)
            wm_s = sb("wm", [P, WM_COLS], bf16)
            vall_s = sb("vall", [P, NKT * 1024], bf16)   # per kt: v(256)|vec(768)
            vq16m_s = sb("vq16m", [P, 6 * NQ], bf16)
            vqAB_s = sb("vqAB", [P, 4 * NQ], bf16)
            vqC2_s = sb("vqC2", [P, 2 * NQ], bf16)
            bm_s = sb("bm", [P, BM_COLS], f32)

            xsT_s = [xm_s[:, i * N:(i + 1) * N] for i in range(2)]
            # query tokens are the first NQ columns of each xsT half
            xqT_s = [xsT_s[i][:, 0:NQ] for i in range(2)]
            vq16_s = [vq16m_s[:, i * NQ:(i + 1) * NQ] for i in range(6)]

            _w = [0]
            def wsl(width):
                o = _w[0]; _w[0] += width
                return wm_s[:, o:o + width]
            wq_s = [wsl(HID) for _ in range(2)]
            wk_s = [wsl(HID) for _ in range(2)]
            wv_s = [wsl(HID) for _ in range(2)]
            wvec_s = [wsl(2 * HID) for _ in range(2)]
            woP2_s = [wsl(3 * HID) for _ in range(2)]
            wgP_s = [wsl(512) for _ in range(4)]     # per pair: 4ic x 128dup
            ones_s = wsl(P)

            bq_s = [bm_s[:, i:i + 1] for i in range(2)]
            bk_s = [bm_s[:, 2 + i:3 + i] for i in range(2)]
            bo_s = [bm_s[:, 4 + i:5 + i] for i in range(6)]
            bgP_s = [bm_s[:, 10 + i:11 + i] for i in range(4)]
            bvB4_s = bm_s[:, 14:14 + 1024]

            kT_s = [sb(f"kT{i}", [P, N], bf16) for i in range(2)]
            qT_s = [sb(f"qT{i}", [P, NQ], bf16) for i in range(2)]
            dot_s = [sb(f"dot{i}", [P, NQ], bf16) for i in range(2)]
            normM_s = sb("normM", [P, 2 * NQ], bf16)
            norm_s = [normM_s[:, i * NQ:(i + 1) * NQ] for i in range(2)]
            gateP_s = [sb(f"gateP{p}", [P, NQ], bf16) for p in range(4)]
            xoutP2_s = [sb(f"xoutP2{i}", [P, NQ], bf16) for i in range(2)]

            # ---------------- PSUM ----------------
            accA = nc.alloc_psum_tensor("accA", [P, 512], f32).ap()
            accB = nc.alloc_psum_tensor("accB", [P, 512], f32).ap()
            accD = nc.alloc_psum_tensor("accD", [P, 512], f32).ap()

            dma = nc.sync.dma_start
            mm = nc.tensor.matmul

            # ---------------- input DMAs (Sync queue, need-ordered) ----------
            vall4 = vall_s.rearrange("p (t c) -> p t c", t=NKT)
            vvec3 = vvec.rearrange("p (t c) -> p t c", t=NKT)
            xs_src = xm.rearrange("p (i c) -> p i c", i=2)
            xs_dst = xm_s.rearrange("p (i c) -> p i c", i=2)
            # critical chunk: weights + first keys (= queries, rotated first)
            dma(out=wm_s[:, 0:1536], in_=wm[:, 0:1536])        # wq|wk|wv
            dma(out=xs_dst[:, :, 0:512], in_=xs_src[:, :, 0:512])
            dma(out=bm_s, in_=bm)
            dma(out=xs_dst[:, :, 512:1024], in_=xs_src[:, :, 512:1024])
            dma(out=vall4[:, 0:4, 256:1024], in_=vvec3[:, 0:4])
            dma(out=vall4[:, 4:8, 256:1024], in_=vvec3[:, 4:8])
            dma(out=xs_dst[:, :, 1024:2048], in_=xs_src[:, :, 1024:2048])
            dma(out=vall4[:, 8:12, 256:1024], in_=vvec3[:, 8:12])
            dma(out=vall4[:, 12:16, 256:1024], in_=vvec3[:, 12:16])
            dma(out=vq16m_s, in_=vq16m)
            dma(out=wm_s[:, 1536:], in_=wm[:, 1536:])
            dma(out=vqAB_s, in_=vqAB)
            dma(out=vqC2_s, in_=vqC2)

            ssP = ctx.enter_context(
                tc.tile_pool(name="ssP", bufs=2, space="PSUM"))
            psDef = ctx.enter_context(
                tc.tile_pool(name="psDef", bufs=1, space="PSUM"))
            exP = ctx.enter_context(tc.tile_pool(name="exP", bufs=4))
            rcP = ctx.enter_context(tc.tile_pool(name="rcP", bufs=2))
            vaP = ctx.enter_context(tc.tile_pool(name="vaP", bufs=5))
            v2P = ctx.enter_context(tc.tile_pool(name="v2P", bufs=5))
            cbP = ctx.enter_context(tc.tile_pool(name="cbP", bufs=2))
            vpP = ctx.enter_context(tc.tile_pool(name="vpP", bufs=2))
            epP = ctx.enter_context(tc.tile_pool(name="epP", bufs=2))
            ntP = ctx.enter_context(tc.tile_pool(name="ntP", bufs=1))

            # ---------------- projection emitters ----------------
            def emit_kT(i, h, eng=None, split=False):
                ps = ssP.tile([P, 1024], f32, tag="ss", name="psk")
                for n in range(2):
                    for ic in range(2):
                        mm(ps[:, ts(n, 512)], wk_s[ic][:, ts(i, P)],
                           xsT_s[ic][:, h * 1024 + n * 512:
                                      h * 1024 + (n + 1) * 512],
                           start=(ic == 0), stop=(ic == 1))
                    if split:
                        (eng or nc.any).tensor_scalar(
                            out=kT_s[i][:, h * 1024 + n * 512:
                                        h * 1024 + (n + 1) * 512],
                            in0=ps[:, ts(n, 512)],
                            scalar1=bk_s[i], scalar2=None, op0=OP.add)
                if not split:
                    (eng or nc.any).tensor_scalar(
                        out=kT_s[i][:, h * 1024:(h + 1) * 1024], in0=ps,
                        scalar1=bk_s[i], scalar2=None, op0=OP.add)

            def emit_qT(i, eng=None, split=False):
                ps = ssP.tile([P, 1024], f32, tag="ss", name="psq")
                for n in range(2):
                    for ic in range(2):
                        mm(ps[:, ts(n, 512)], wq_s[ic][:, ts(i, P)],
                           xqT_s[ic][:, ts(n, 512)],
                           start=(ic == 0), stop=(ic == 1))
                    if split:
                        (eng or nc.any).tensor_scalar(
                            out=qT_s[i][:, ts(n, 512)], in0=ps[:, ts(n, 512)],
                            scalar1=bq_s[i], scalar2=None, op0=OP.add)
                if not split:
                    (eng or nc.any).tensor_scalar(
                        out=qT_s[i], in0=ps,
                        scalar1=bq_s[i], scalar2=None, op0=OP.add)

            def emit_vproj(g):
                ps = ssP.tile([P, 1024], f32, tag="ss", name="psv")
                for t4 in range(4):
                    t = 4 * g + t4
                    for ic in range(2):
                        mm(ps[:, ts(t4, 256)], xsT_s[ic][:, ts(t, P)], wv_s[ic],
                           start=(ic == 0), stop=(ic == 1))
                vdst = vall_s[:, g * 4096:g * 4096 + 4096]
                vd3 = vdst.rearrange("p (t c) -> p t c", t=4)
                bv3 = bvB4_s.rearrange("p (t c) -> p t c", t=4)
                nc.vector.tensor_tensor(out=vd3[:, :, 0:256], in0=ps.rearrange(
                    "p (t c) -> p t c", t=4), in1=bv3, op=OP.add)

            def emit_kT_half(i, h, n, eng=None):
                ps = ssP.tile([P, 1024], f32, tag="ss", name="psk")
                for ic in range(2):
                    mm(ps[:, ts(n, 512)], wk_s[ic][:, ts(i, P)],
                       xsT_s[ic][:, h * 1024 + n * 512:
                                  h * 1024 + (n + 1) * 512],
                       start=(ic == 0), stop=(ic == 1))
                (eng or nc.any).tensor_scalar(
                    out=kT_s[i][:, h * 1024 + n * 512:
                                h * 1024 + (n + 1) * 512],
                    in0=ps[:, ts(n, 512)],
                    scalar1=bk_s[i], scalar2=None, op0=OP.add)

            def emit_qT_half(i, n, eng=None):
                ps = ssP.tile([P, 1024], f32, tag="ss", name="psq")
                for ic in range(2):
                    mm(ps[:, ts(n, 512)], wq_s[ic][:, ts(i, P)],
                       xqT_s[ic][:, ts(n, 512)],
                       start=(ic == 0), stop=(ic == 1))
                (eng or nc.any).tensor_scalar(
                    out=qT_s[i][:, ts(n, 512)], in0=ps[:, ts(n, 512)],
                    scalar1=bq_s[i], scalar2=None, op0=OP.add)

            # minimal pre-phase: just enough for iter 0 kt 0..3
            emit_kT_half(0, 0, 0)
            emit_qT_half(0, 0)
            emit_vproj(0)

            def gen_projrest():
                v = nc.vector
                emit_kT_half(0, 0, 1, v); yield
                emit_kT_half(0, 1, 0, v); yield
                emit_kT_half(0, 1, 1, v); yield
                emit_vproj(1); yield
                emit_vproj(2); yield
                emit_vproj(3); yield
                emit_qT_half(0, 1, v); yield
                emit_kT_half(1, 0, 0, v); yield
                emit_kT_half(1, 0, 1, v); yield
                emit_kT_half(1, 1, 0, v); yield
                emit_kT_half(1, 1, 1, v); yield
                emit_qT_half(1, 0, v); yield
                emit_qT_half(1, 1, v); yield

            # ---------------- attention + deferred work ----------------
            # deferred emission generators: each yields small chunks
            def gen_vec_proj():
                # vec_proj pairs -> dot ; emits per (c, n-half, o-tile)
                vp_sb = [None] * 4
                step = [0]
                for c in range(3):
                    for n in range(2):
                        for o in range(4):
                            ps = psDef.tile([P, 512], f32, tag="d", name="vp")
                            with tc.tile_wait_until(0.020 + step[0] * 0.0008):
                                for ic in range(2):
                                    mm(ps, wvec_s[ic][:, ts(o, P)],
                                       vq16_s[2 * c + ic][:, ts(n, 512)],
                                       start=(ic == 0), stop=(ic == 1))
                                t = vpP.tile([P, 512], bf16, tag=f"vp{o}",
                                             name=f"vp{o}")
                                nc.vector.tensor_copy(t, ps)
                            vp_sb[o] = t
                            step[0] += 1
                            yield
                        for jj in range(2):
                            dsl = dot_s[jj][:, ts(n, 512)]
                            if c == 0:
                                nc.vector.tensor_tensor(
                                    out=dsl, in0=vp_sb[jj], in1=vp_sb[2 + jj],
                                    op=OP.mult)
                            else:
                                m2 = vpP.tile([P, 512], bf16, tag="dt",
                                              name="dt")
                                nc.vector.tensor_tensor(
                                    out=m2, in0=vp_sb[jj], in1=vp_sb[2 + jj],
                                    op=OP.mult)
                                nc.vector.tensor_tensor(
                                    out=dsl, in0=dsl, in1=m2, op=OP.add)
                        yield

            def gen_norm():
                # nt = sum vec^2 (DVE squares, GpSimd/DVE adds), then ONE
                # Sqrt ACT over both halves (single table switch out+back)
                ntM = ntP.tile([P, 2 * NQ], bf16, tag="ntM", name="ntM")
                for jj in range(2):
                    nt = ntM[:, jj * NQ:(jj + 1) * NQ]
                    nc.vector.tensor_tensor(out=nt, in0=vq16_s[jj],
                                            in1=vq16_s[jj], op=OP.mult)
                    yield
                    for c in (1, 2):
                        m2 = ntP.tile([P, NQ], bf16, tag="nt2", name="nt2")
                        nc.vector.tensor_tensor(out=m2, in0=vq16_s[2 * c + jj],
                                                in1=vq16_s[2 * c + jj],
                                                op=OP.mult)
                        yield
                        nc.vector.tensor_tensor(out=nt, in0=nt, in1=m2,
                                                op=OP.add)
                        yield
                with tc.tile_wait_until(0.048):
                    nc.scalar.activation(normM_s, ntM, AF.Sqrt)
                yield

            def gen_gate():
                # gateP[p] = 1/(1+exp(-(z+bg))) duplicated over both bands
                inv = [dot_s[0], dot_s[1], norm_s[0], norm_s[1]]
                for p in range(4):
                    for n in range(2):
                        with tc.tile_wait_until(0.051 + (2 * p + n) * 0.0012):
                            ps = psDef.tile([P, 512], f32, tag="d", name="gz")
                            for ic in range(4):
                                mm(ps, wgP_s[p][:, ts(ic, P)],
                                   inv[ic][:, ts(n, 512)],
                                   start=(ic == 0), stop=(ic == 3))
                            e = cbP.tile([P, 512], f32, tag="ge", name="ge")
                            nc.scalar.activation(e, ps, AF.Exp, bias=bgP_s[p],
                                                 scale=-1.0)
                            nc.vector.tensor_scalar(out=e, in0=e, scalar1=1.0,
                                                    scalar2=None, op0=OP.add)
                            g32 = cbP.tile([P, 512], f32, tag="g32",
                                           name="g32")
                            nc.vector.reciprocal_approx_fast(out=g32, in_=e)
                            nc.vector.tensor_copy(
                                gateP_s[p][:, ts(n, 512)], g32)
                        if n == 1:
                            gate_emitted[p] = True
                        yield

            def gen_epilogue(n):
                # x_updated[:, n*512:+512] ; needs xoutP2 cols, dot, norm
                for j in range(2):
                    xu = epP.tile([P, 512], bf16, tag="xu", name="xu")
                    for k in (2, 0, 1):   # o3 first, then o1*dot, o2*norm
                        ps = psDef.tile([P, 512], f32, tag="d", name="eo")
                        for pp in range(2):
                            mm(ps, woP2_s[pp][:, 256 * k + 128 * j:
                                              256 * k + 128 * (j + 1)],
                               xoutP2_s[pp][:, ts(n, 512)],
                               start=(pp == 0), stop=(pp == 1))
                        if k == 2:
                            nc.vector.tensor_scalar(
                                out=xu, in0=ps, scalar1=bo_s[4 + j],
                                scalar2=None, op0=OP.add)
                        else:
                            src = dot_s[j] if k == 0 else norm_s[j]
                            t = epP.tile([P, 512], bf16, tag="et", name="et")
                            nc.vector.scalar_tensor_tensor(
                                out=t, in0=ps, scalar=bo_s[2 * k + j],
                                in1=src[:, ts(n, 512)],
                                op0=OP.add, op1=OP.mult)
                            nc.vector.tensor_tensor(out=xu, in0=xu, in1=t,
                                                    op=OP.add)
                        yield
                    dma(out=out[j * P:(j + 1) * P, ts(n, 512)], in_=xu)
                    yield

            deferred = []
            deferred += list()  # placeholder

            # build the deferred schedule as a flat list of generators
            def chain(*gens):
                for g in gens:
                    yield from g

            def gen_spacer(k):
                for _ in range(k):
                    yield

            defer_iter = chain(gen_projrest(), gen_spacer(8),
                               gen_vec_proj(), gen_norm(), gen_gate())
            defer_done = [False]

            def emit_defer(k):
                if defer_done[0]:
                    return
                for _ in range(k):
                    try:
                        next(defer_iter)
                    except StopIteration:
                        defer_done[0] = True
                        return

            pending_combines = []

            gate_emitted = [False] * 4

            def emit_combines(limit=99, cur_it=99):
                k = 0
                while (pending_combines and k < limit
                       and gate_emitted[pending_combines[0][0]]
                       and cur_it - pending_combines[0][1] >= 2):
                    pending_combines.pop(0)[2]()
                    k += 1

            def iter_params(it):
                p, qh = it // 2, it % 2
                j, prw = p // 2, 64 * (p % 2)
                swap = p % 2
                xo_r, v2_r = (0, 64) if swap == 0 else (64, 0)
                qsl = slice(qh * 512, (qh + 1) * 512)
                return p, qh, j, prw, xo_r, v2_r, qsl

            def emit_scores(it, kt):
                p, qh, j, prw, xo_r, v2_r, qsl = iter_params(it)
                ss = ssP.tile([P, 1024], f32, tag="ss", name="ss")
                for m in range(2):
                    rb = prw + 32 * m
                    mm(ss[:, ts(m, 512)],
                       kT_s[j][rb:rb + 32, ts(kt, P)],
                       qT_s[j][rb:rb + 32, qsl],
                       start=True, stop=True, tile_position=(rb, 0))
                return ss

            def emit_pv(it, kt, ex):
                p, qh, j, prw, xo_r, v2_r, qsl = iter_params(it)
                first, last = kt == 0, kt == NKT - 1
                vbase = kt * 1024
                for m in range(2):
                    exm = ex[:, ts(m, 512)]
                    mm(accD[32 * m:32 * m + 32, :], ones_s[:, 0:HD],
                       exm, start=first, stop=last,
                       tile_position=(0, 32 * m))
                    mm(accD[64 + 32 * m:64 + 32 * m + 32, :],
                       ones_s[:, 0:HD], exm, start=first, stop=last,
                       tile_position=(0, 64 + 32 * m))
                for m in range(2):
                    h = 2 * p + m
                    exm = ex[:, ts(m, 512)]
                    mm(accB[32 * m:32 * m + 32, :],
                       vall_s[:, vbase + 256 + 32 * h:vbase + 256 + 32 * h + 32],
                       exm, start=first, stop=last,
                       tile_position=(0, 32 * m))
                    mm(accB[64 + 32 * m:64 + 32 * m + 32, :],
                       vall_s[:, vbase + 512 + 32 * h:vbase + 512 + 32 * h + 32],
                       exm, start=first, stop=last,
                       tile_position=(0, 64 + 32 * m))
                for m in range(2):
                    h = 2 * p + m
                    exm = ex[:, ts(m, 512)]
                    mm(accA[xo_r + 32 * m:xo_r + 32 * m + 32, :],
                       vall_s[:, vbase + 32 * h:vbase + 32 * h + 32],
                       exm, start=first, stop=last,
                       tile_position=(0, xo_r + 32 * m))
                    mm(accA[v2_r + 32 * m:v2_r + 32 * m + 32, :],
                       vall_s[:, vbase + 768 + 32 * h:vbase + 768 + 32 * h + 32],
                       exm, start=first, stop=last,
                       tile_position=(0, v2_r + 32 * m))

            def emit_iter_end(it):
                p, qh, j, prw, xo_r, v2_r, qsl = iter_params(it)
                rc = rcP.tile([P, 512], f32, tag="rc", name="rc")
                nc.vector.reciprocal_approx_fast(out=rc, in_=accD)
                mixA = v2P.tile([P, 512], bf16, tag="v2", name="v2")
                nc.vector.tensor_tensor(out=mixA, in0=accA, in1=rc,
                                        op=OP.mult)
                vaR = vaP.tile([P, 512], bf16, tag="va", name="va")
                nc.vector.tensor_tensor(out=vaR, in0=accB, in1=rc, op=OP.mult)
                if it == 7:
                    nc.vector.tensor_tensor(
                        out=xoutP2_s[p // 2][xo_r:xo_r + 64, qsl],
                        in0=accA[xo_r:xo_r + 64, :],
                        in1=rc[xo_r:xo_r + 64, :], op=OP.mult)
                else:
                    nc.gpsimd.tensor_copy(
                        xoutP2_s[p // 2][xo_r:xo_r + 64, qsl],
                        mixA[xo_r:xo_r + 64, :])
                va2R = mixA

                def go(p=p, qh=qh, qsl=qsl, vaR=vaR, va2R=va2R, v2_r=v2_r):
                    t1 = cbP.tile([P, 512], bf16, tag="c1", name="c1")
                    nc.vector.tensor_tensor(out=t1, in0=vaR,
                                            in1=gateP_s[p][:, qsl],
                                            op=OP.mult)
                    ab0 = p * NQ + qsl.start
                    nc.vector.tensor_tensor(
                        out=t1, in0=t1, in1=vqAB_s[:, ab0:ab0 + 512],
                        op=OP.add)
                    dma(out=out[256 + 64 * p:256 + 64 * p + 64, qsl],
                        in_=t1[0:64, :])
                    dma(out=out[512 + 64 * p:512 + 64 * p + 64, qsl],
                        in_=t1[64:128, :])
                    t2 = cbP.tile([P, 512], bf16, tag="c2", name="c2")
                    nc.vector.tensor_tensor(
                        out=t2[v2_r:v2_r + 64, :],
                        in0=va2R[v2_r:v2_r + 64, :],
                        in1=gateP_s[p][v2_r:v2_r + 64, qsl], op=OP.mult)
                    c20 = (p // 2) * NQ + qsl.start
                    nc.vector.tensor_tensor(
                        out=t2[v2_r:v2_r + 64, :],
                        in0=t2[v2_r:v2_r + 64, :],
                        in1=vqC2_s[v2_r:v2_r + 64, c20:c20 + 512],
                        op=OP.add)
                    dma(out=out[768 + 64 * p:768 + 64 * p + 64, qsl],
                        in_=t2[v2_r:v2_r + 64, :])
                pending_combines.append((p, it, go))

            # ---- global pipelined attention: 8 iters x 16 kt ----
            # PV lags two steps behind exp so the boundary normalize chain
            # (WAR on the accumulators) never sits in front of the next
            # iteration's score matmuls in the PE queue.
            steps = [(it, kt) for it in range(8) for kt in range(NKT)]
            ss_cur = emit_scores(*steps[0])
            pvq = []

            epi0 = [None]

            def flush_one():
                fit, fkt, fex = pvq.pop(0)
                emit_pv(fit, fkt, fex)
                if fkt == NKT - 1:
                    emit_iter_end(fit)
                    if fit == 6:
                        epi0[0] = gen_epilogue(0)
                elif fkt in (5, 10):
                    emit_combines(limit=1, cur_it=fit)
                if epi0[0] is not None:
                    try:
                        next(epi0[0])
                    except StopIteration:
                        epi0[0] = None

            for idx, (it, kt) in enumerate(steps):
                ex = exP.tile([P, 1024], bf16, tag="ex", name="ex")
                nc.scalar.activation(ex, ss_cur, AF.Exp)
                if idx + 1 < len(steps):
                    ss_nxt = emit_scores(*steps[idx + 1])
                else:
                    ss_nxt = None
                if len(pvq) == 2:
                    flush_one()
                pvq.append((it, kt, ex))
                ss_cur = ss_nxt
                emit_defer(1)
            while pvq:
                flush_one()
            while not defer_done[0]:
                emit_defer(8)
            while pending_combines:
                pending_combines.pop(0)[2]()

            # final epilogue (n=1) on freed ss-pool banks: parallel chunks
            ep_tiles = []
            for k in range(3):
                t = ssP.tile([P, 1024], f32, tag="ss", name=f"ep{k}")
                for j in range(2):
                    for pp in range(2):
                        mm(t[:, ts(j, 512)],
                           woP2_s[pp][:, 256 * k + 128 * j:
                                      256 * k + 128 * (j + 1)],
                           xoutP2_s[pp][:, 512:1024],
                           start=(pp == 0), stop=(pp == 1))
                ep_tiles.append(t)
            for j in range(2):
                xu = epP.tile([P, 512], bf16, tag="xu", name="xu")
                nc.any.tensor_scalar(out=xu, in0=ep_tiles[2][:, ts(j, 512)],
                                     scalar1=bo_s[4 + j], scalar2=None,
                                     op0=OP.add)
                t = epP.tile([P, 512], bf16, tag="et", name="et")
                nc.vector.scalar_tensor_tensor(
                    out=t, in0=ep_tiles[0][:, ts(j, 512)], scalar=bo_s[j],
                    in1=dot_s[j][:, 512:1024], op0=OP.add, op1=OP.mult)
                nc.vector.tensor_tensor(out=xu, in0=xu, in1=t, op=OP.add)
                t2 = epP.tile([P, 512], bf16, tag="et2", name="et2")
                nc.vector.scalar_tensor_tensor(
                    out=t2, in0=ep_tiles[1][:, ts(j, 512)], scalar=bo_s[2 + j],
                    in1=norm_s[j][:, 512:1024], op0=OP.add, op1=OP.mult)
                nc.vector.tensor_tensor(out=xu, in0=xu, in1=t2, op=OP.add)
                dma(out=out[j * P:(j + 1) * P, 512:1024], in_=xu)

    nc.compile()
    return nc


def _get_nc():
    if "nc" not in _CACHE:
        _CACHE["nc"] = _build_nc()
    return _CACHE["nc"]


def _make_in_maps(inputs):
    x = np.asarray(inputs["x"], np.float32)
    Wq = np.asarray(inputs["Wq"], np.float32)
    Wk = np.asarray(inputs["Wk"], np.float32)
    Wv = np.asarray(inputs["Wv"], np.float32)
    Wvec = np.asarray(inputs["Wvec"], np.float32)
    Wo = np.asarray(inputs["Wo"], np.float32)
    Wg = np.asarray(inputs["Wg"], np.float32)
    bq = np.asarray(inputs["bq"], np.float32)
    bk = np.asarray(inputs["bk"], np.float32)
    bv = np.asarray(inputs["bv"], np.float32)
    bo = np.asarray(inputs["bo"], np.float32)
    bg = np.asarray(inputs["bg"], np.float32)
    a_d = float(np.asarray(inputs["alpha_dot"]))
    a_n = float(np.asarray(inputs["alpha_norm"]))

    wgT = Wg.T.copy()            # [512, 256]
    wgT[:HID, :] *= a_d
    wgT[HID:, :] *= a_n

    wqs = Wq.T * SCALE           # fold score scale into Wq (and bq below)
    # wgP: per pair p, 4 ic tiles [128, 128] with cols = ch 64p..64p+64 dup x2
    wgP = []
    for p in range(4):
        cols = wgT[:, 64 * p:64 * p + 64]
        dup = np.concatenate([cols, cols], axis=1)   # [512, 128]
        wgP.append(np.concatenate([dup[128 * ic:128 * (ic + 1)]
                                   for ic in range(4)], axis=1))  # [128, 512]
    # woP2: pair-pairs stacked on partitions: [128 = p_even|p_odd, 768]
    woT = Wo.T                   # [256, 768]
    woP2 = [np.concatenate([woT[128 * i:128 * i + 64],
                            woT[128 * i + 64:128 * (i + 1)]], axis=0)
            for i in range(2)]
    # note: rows 0:64 = pair 2i (even), rows 64:128 = pair 2i+1 (odd)

    wmh = np.concatenate([
        wqs[0:128], wqs[128:256], Wk.T[0:128], Wk.T[128:256],
        Wv.T[0:128], Wv.T[128:256], Wvec.T[0:128], Wvec.T[128:256],
        woP2[0], woP2[1],
        wgP[0], wgP[1], wgP[2], wgP[3],
        np.ones((128, 128), np.float32)], axis=1)

    bmh = np.zeros((128, 1038), np.float32)
    for i in range(2):
        bmh[:, i] = bq[i * 128:(i + 1) * 128] * SCALE
        bmh[:, 2 + i] = bk[i * 128:(i + 1) * 128]
    for i in range(6):
        bmh[:, 4 + i] = bo[i * 128:(i + 1) * 128]
    for p in range(4):
        bgp = bg[64 * p:64 * p + 64]
        bmh[:, 10 + p] = -np.concatenate([bgp, bgp])
    bmh[:, 14:14 + 1024] = np.tile(np.broadcast_to(bv, (128, HID)), (1, 4))

    common = {
        "wm": np.ascontiguousarray(wmh).astype(BF),
        "bm": np.ascontiguousarray(bmh),
    }

    in_maps = []
    for core in range(8):
        b, qhl = core // 2, core % 2
        qs = slice(qhl * NQ, (qhl + 1) * NQ)
        rot = np.r_[qhl * NQ:(qhl + 1) * NQ, 0:qhl * NQ, (qhl + 1) * NQ:N]
        xsT = np.ascontiguousarray(x[b, rot, 0, :].T)        # [256, 2048]
        vq = x[b, qs, 1:, :].transpose(1, 2, 0).reshape(3 * HID, NQ)
        vq6 = np.concatenate([vq[i * 128:(i + 1) * 128] for i in range(6)],
                             axis=1)
        vkv_t = x[b, rot, 1:, :].reshape(N, 3 * HID)
        vvec_h = np.concatenate([vkv_t[t * 128:(t + 1) * 128]
                                 for t in range(NKT)], axis=1)
        xmh = np.concatenate([xsT[0:128], xsT[128:256]], axis=1)
        # vqAB: per pair p: [c0 ch(64p..) ; c1 ch(64p..)] stacked rows
        vAB = np.empty((128, 4 * NQ), np.float32)
        for p in range(4):
            vAB[0:64, p * NQ:(p + 1) * NQ] = vq[0:256][64 * p:64 * p + 64]
            vAB[64:128, p * NQ:(p + 1) * NQ] = vq[256:512][64 * p:64 * p + 64]
        # vqC2: per pair-pair i: even pair p=2i at rows 64:128 (va2 band),
        # odd pair p=2i+1 at rows 0:64
        vC2 = np.empty((128, 2 * NQ), np.float32)
        for i in range(2):
            pe, po = 2 * i, 2 * i + 1
            vC2[64:128, i * NQ:(i + 1) * NQ] = \
                vq[512:768][64 * pe:64 * pe + 64]
            vC2[0:64, i * NQ:(i + 1) * NQ] = vq[512:768][64 * po:64 * po + 64]
        m = dict(common)
        m["xm"] = np.ascontiguousarray(xmh).astype(BF)
        m["vq16m"] = np.ascontiguousarray(vq6).astype(BF)
        m["vvec"] = np.ascontiguousarray(vvec_h).astype(BF)
        m["vqAB"] = np.ascontiguousarray(vAB).astype(BF)
        m["vqC2"] = np.ascontiguousarray(vC2).astype(BF)
        in_maps.append(m)
    return in_maps


def _gather(results):
    x_final = np.empty((B, N, 4, HID), np.float32)
    for core, res in enumerate(results):
        b, qhl = core // 2, core % 2
        qs = slice(qhl * NQ, (qhl + 1) * NQ)
        o = np.asarray(res["out"], np.float32)   # [1024 ch, 1024 q] (bf16)
        for c in range(4):
            x_final[b, qs, c, :] = o[c * HID:(c + 1) * HID, :].T
    return x_final


def _run(inputs, trace=False):
    from concourse.bass_utils import run_bass_kernel_spmd
    nc = _get_nc()
    in_maps = _make_in_maps(inputs)
    res = run_bass_kernel_spmd(nc, in_maps, core_ids=list(range(8)),
                               trace=trace)
    return _gather(res.results), res


def kernel(**inputs):
    out, _ = _run(inputs, trace=False)
    return out


def _install_trace_hook():
    import types
    try:
        import antenv.axon_hooks as ah
    except ModuleNotFoundError:
        import antenv
        ah = types.ModuleType("antenv.axon_hooks")
        _hook = [None]
        ah.set_axon_ntff_profile_hook = lambda h: _hook.__setitem__(0, h)
        ah.get_axon_ntff_profile_hook = lambda: _hook[0]
        sys.modules["antenv.axon_hooks"] = ah
        antenv.axon_hooks = ah
    if ah.get_axon_ntff_profile_hook() is None:
        if "/root/.axon_site" not in sys.path:
            sys.path.insert(0, "/root/.axon_site")
        from trn_agent_boot.trn_boot import _ntff_profile_via_ctypes
        ah.set_axon_ntff_profile_hook(
            _ntff_profile_via_ctypes("/opt/axon/libaxon_pjrt.so"))
    import concourse.bass_utils as bu
    bu.upload_artifacts = lambda tmpdir: tmpdir


def run_traced(inputs, tmpdir=None):
    _install_trace_hook()
    from concourse.bass_utils import run_bass_kernel_spmd
    nc = _get_nc()
    in_maps = _make_in_maps(inputs)
    res = run_bass_kernel_spmd(nc, in_maps, core_ids=list(range(8)),
                               trace=True, tmpdir=tmpdir)
    return _gather(res.results), res
